# revision 6
# baseline (speedup 1.0000x reference)
"""Trainium2 Bass kernel for the MD5-surrogate stacked-MLP problem.

Contract: kernel(**inputs) takes the FULL unsharded inputs
(message_bytes (4096,64), W1 (64,512,162), b1 (64,512), W2 (64,512,512),
b2 (64,512), W3 (64,128,512), b3 (64,128)) and returns the full (4096,128)
output. Batch is sharded across 8 NeuronCores (512 rows each); the 64
sequential round-MLPs run locally per shard (weights replicated).

On-chip layout: activations are [feature, batch] = [partitions, free].
All matmuls run as float32r (full-rate 1 cycle/row for N>=256).

Structural trick: the inter-round state is linear (no activation), so
W3[r] is folded into round r+1's first layer on the host:
    M1[r] = W1[r+1][:, :128] @ W3[r]          (512x512)
so each round r>=1 computes
    h1 = gelu(M1[r-1] @ h2_prev + W1x @ word_r + b1_eff)
    h2 = gelu(W2 @ h1 + b2)
with no mm3 / state copy / cross-round serial bridge. Only round 63
computes the actual state output via W3[63].

Biases: b1_eff (b1 + rinfo terms + W1s @ b3[r-1], and W1s @ INIT for r=0)
and b2 ride the per-partition bias port of the gelu ACTIVATE. b3[63] is
applied in the final Identity copy.

The 32 word bits use 4 row-tiled K=32 matmuls (tile_position=(32m,0))
running concurrently in the PE array, with the word tile replicated
across 4 partition strips.
"""

import numpy as np
from contextlib import ExitStack

import concourse.bass as bass
import concourse.mybir as mybir
import concourse.tile as tile
from concourse import bacc, bass_utils

F32 = mybir.dt.float32
F32R = mybir.dt.float32r
AF = mybir.ActivationFunctionType

B = 4096
NCORES = 8
BS = B // NCORES  # 512 rows per core
NR = 64
DH = 512

# ---- MD5 schedule constants (mirrors reference.py) ----
_SCHED = np.array(
    [i if i < 16 else (5 * i + 1) % 16 if i < 32 else (3 * i + 5) % 16 if i < 48 else (7 * i) % 16
     for i in range(64)], dtype=np.int32)
_BASES = [[7, 12, 17, 22], [5, 9, 14, 20], [4, 11, 16, 23], [6, 10, 15, 21]]
_SHIFT = np.array([_BASES[i // 16][i % 4] for i in range(64)], dtype=np.float32)
_RINFO = np.stack([np.arange(64, dtype=np.float32) / 64.0, _SHIFT / 25.0], axis=1)
_INIT_WORDS = np.array([1732584193, 4023233417, 2562383102, 271733878], dtype=np.int64)
_INIT_BITS = (((_INIT_WORDS[:, None] >> np.arange(32)) & 1).astype(np.float32)).reshape(128)

_CACHED_NC = None


def _build_kernel():
    nc = bacc.Bacc(trn_type="TRN2", target_bir_lowering=False, debug=False)

    m1p = nc.dram_tensor("m1p", [NR - 1, 128, 4 * DH], F32R, kind="ExternalInput").ap()
    w1x = nc.dram_tensor("w1x", [128, NR * 128], F32R, kind="ExternalInput").ap()
    w2p = nc.dram_tensor("w2p", [NR, 128, 4 * DH], F32R, kind="ExternalInput").ap()
    w3l = nc.dram_tensor("w3l", [128, DH], F32R, kind="ExternalInput").ap()
    bbp = nc.dram_tensor("bbp", [128, NR * 8], F32, kind="ExternalInput").ap()
    xtra = nc.dram_tensor("xtra", [NR, 128, BS], F32R, kind="ExternalInput").ap()
    b3l = nc.dram_tensor("b3l", [128, 1], F32, kind="ExternalInput").ap()
    out = nc.dram_tensor("out", [128, BS], F32, kind="ExternalOutput").ap()

    with ExitStack() as ctx:
        tc = ctx.enter_context(tile.TileContext(nc))
        wpool = ctx.enter_context(tc.tile_pool(name="w", bufs=3))
        xpool = ctx.enter_context(tc.tile_pool(name="x", bufs=3))
        hpool = ctx.enter_context(tc.tile_pool(name="h", bufs=2))
        bpool = ctx.enter_context(tc.tile_pool(name="b", bufs=3))
        opool = ctx.enter_context(tc.tile_pool(name="o", bufs=1))
        cpool = ctx.enter_context(tc.tile_pool(name="c", bufs=1))
        ps = ctx.enter_context(tc.tile_pool(name="ps", bufs=8, space="PSUM"))

        w1x_all = cpool.tile([128, NR * 128], F32R)
        nc.sync.dma_start(w1x_all[:], w1x[:])
        bb_all = cpool.tile([128, NR * 8], F32)
        nc.sync.dma_start(bb_all[:], bbp[:])

        h2_prev = None
        for r in range(NR):
            # ---- weight / input DMAs for this round ----
            xw_t = xpool.tile([128, BS], F32R, tag="x")
            nc.sync.dma_start(xw_t[:], xtra[r])
            if r > 0:
                m1_t = wpool.tile([128, 4 * DH], F32R, tag="m1")
                nc.sync.dma_start(m1_t[:], m1p[r - 1])
            w2_t = wpool.tile([128, 4 * DH], F32R, tag="w2")
            nc.sync.dma_start(w2_t[:], w2p[r])
            w1x_t = w1x_all[:, 128 * r : 128 * (r + 1)]
            bb_t = bb_all[:, 8 * r : 8 * (r + 1)]

            # ---- layer 1: h1 = gelu(M1 @ h2_prev + W1x @ word + b1_eff) ----
            pb1 = [ps.tile([128, BS], F32, tag="pb", name=f"pb1_{r}_{m}") for m in range(4)]
            for m in range(4):
                nc.tensor.matmul(pb1[m][:], w1x_t[32 * m : 32 * (m + 1), :],
                                 xw_t[32 * m : 32 * (m + 1), :],
                                 start=True, stop=(r == 0),
                                 tile_position=(32 * m, 0))
            if r > 0:
                for k in range(4):
                    for m in range(4):
                        nc.tensor.matmul(
                            pb1[m][:],
                            m1_t[:, DH * k + 128 * m : DH * k + 128 * (m + 1)],
                            h2_prev[:, BS * k : BS * (k + 1)],
                            start=False, stop=(k == 3))
            h1_t = hpool.tile([128, 4 * BS], F32R, tag="h1")
            for m in range(4):
                nc.scalar.activation(h1_t[:, BS * m : BS * (m + 1)], pb1[m][:],
                                     AF.Gelu, bias=bb_t[:, m : m + 1])

            # ---- layer 2: h2 = gelu(W2 @ h1 + b2) ----
            pb2 = [ps.tile([128, BS], F32, tag="pb", name=f"pb2_{r}_{m}") for m in range(4)]
            for k in range(4):  # k-outer: consume h1 chunks as ACT produces them
                for m in range(4):
                    nc.tensor.matmul(
                        pb2[m][:],
                        w2_t[:, DH * k + 128 * m : DH * k + 128 * (m + 1)],
                        h1_t[:, BS * k : BS * (k + 1)],
                        start=(k == 0), stop=(k == 3))
            h2_t = hpool.tile([128, 4 * BS], F32R, tag="h2")
            for m in range(4):
                nc.scalar.activation(h2_t[:, BS * m : BS * (m + 1)], pb2[m][:],
                                     AF.Gelu, bias=bb_t[:, 4 + m : 5 + m])
            h2_prev = h2_t

        # ---- final: out = W3[63] @ h2 + b3[63] ----
        w3_t = wpool.tile([128, DH], F32R, tag="w3")
        nc.sync.dma_start(w3_t[:], w3l[:])
        b3l_t = bpool.tile([128, 1], F32, tag="b3l")
        nc.sync.dma_start(b3l_t[:], b3l[:])
        psum3 = ps.tile([128, BS], F32, tag="pb")
        for k in range(4):
            nc.tensor.matmul(psum3[:], w3_t[:, 128 * k : 128 * (k + 1)],
                             h2_prev[:, BS * k : BS * (k + 1)],
                             start=(k == 0), stop=(k == 3))
        o_t = opool.tile([128, BS], F32)
        nc.scalar.activation(o_t[:], psum3[:], AF.Identity, bias=b3l_t[:])
        nc.sync.dma_start(out[:], o_t[:])

    nc.compile()
    return nc


def _prep_inputs(message_bytes, W1, b1, W2, b2, W3, b3):
    """Host-side packing: bit unpack + schedule + weight transposes + folds."""
    message_bytes = np.asarray(message_bytes, dtype=np.float32)
    W1 = np.asarray(W1, dtype=np.float32)
    b1 = np.asarray(b1, dtype=np.float32)
    W2 = np.asarray(W2, dtype=np.float32)
    b2 = np.asarray(b2, dtype=np.float32)
    W3 = np.asarray(W3, dtype=np.float32)
    b3 = np.asarray(b3, dtype=np.float32)

    mb = np.round(message_bytes * 255.0).astype(np.int32)               # (B,64)
    bits = ((mb[..., None] >> np.arange(8, dtype=np.int32)) & 1).astype(np.float32)
    words = bits.reshape(B, 16, 32)                                      # (B,16,32)
    words_sched = words[:, _SCHED, :]                                    # (B,64,32)
    xtra_full = np.ascontiguousarray(
        np.tile(words_sched.transpose(1, 2, 0), (1, 4, 1)))              # (64,128,B) replicated

    # M1[r] = W1[r+1][:, :128] @ W3[r]  (fp64 for accuracy)
    M1 = np.einsum("rhs,rsk->rhk", W1[1:, :, 0:128].astype(np.float64),
                   W3[:-1].astype(np.float64)).astype(np.float32)        # (63,512,512)
    # m1p[r, p, 512k+128m+j] = M1[r][128m+j, 128k+p]
    m1p = np.ascontiguousarray(
        M1.reshape(NR - 1, 4, 128, 4, 128).transpose(0, 4, 3, 1, 2).reshape(NR - 1, 128, 4 * DH))

    # word weights, stacked for 4-strip row tiling: rows 32m..32m+31 hold
    # the K=32 block for output tile m
    w1xs = (W1[:, :, 128:160].reshape(NR, 4, 128, 32)
            .transpose(0, 1, 3, 2).reshape(NR, 128, 128))
    w1xs = np.ascontiguousarray(w1xs.transpose(1, 0, 2).reshape(128, NR * 128))

    w2pk = np.ascontiguousarray(
        W2.reshape(NR, 4, 128, 4, 128).transpose(0, 4, 3, 1, 2).reshape(NR, 128, 4 * DH))
    w3lp = np.ascontiguousarray(
        W3[NR - 1].reshape(128, 4, 128).transpose(2, 1, 0).reshape(128, DH))       # [p, 128k+m]

    b1_eff = b1 + W1[:, :, 160] * _RINFO[:, 0:1] + W1[:, :, 161] * _RINFO[:, 1:2]
    b1_eff[0] += (W1[0, :, 0:128].astype(np.float64) @ _INIT_BITS.astype(np.float64)).astype(np.float32)
    b1_eff[1:] += np.einsum("rhs,rs->rh", W1[1:, :, 0:128].astype(np.float64),
                            b3[:-1].astype(np.float64)).astype(np.float32)

    bb = np.empty((NR, 128, 8), np.float32)
    bb[:, :, 0:4] = b1_eff.reshape(NR, 4, 128).transpose(0, 2, 1)
    bb[:, :, 4:8] = b2.reshape(NR, 4, 128).transpose(0, 2, 1)
    bb = np.ascontiguousarray(bb.transpose(1, 0, 2).reshape(128, NR * 8))

    b3l = np.ascontiguousarray(b3[NR - 1].reshape(128, 1))

    shared = {"m1p": m1p, "w1x": w1xs, "w2p": w2pk, "w3l": w3lp, "bbp": bb, "b3l": b3l}
    in_maps = []
    for c in range(NCORES):
        m = dict(shared)
        m["xtra"] = np.ascontiguousarray(xtra_full[:, :, c * BS : (c + 1) * BS])
        in_maps.append(m)
    return in_maps


def _run(inputs, trace=False, **kw):
    global _CACHED_NC
    if _CACHED_NC is None:
        _CACHED_NC = _build_kernel()
    in_maps = _prep_inputs(**inputs)
    res = bass_utils.run_bass_kernel_spmd(
        _CACHED_NC, in_maps, core_ids=list(range(NCORES)), trace=trace, **kw)
    outs = [res.results[c]["out"].T for c in range(NCORES)]   # each (BS,128)
    return np.concatenate(outs, axis=0).astype(np.float32), res


def kernel(**inputs) -> np.ndarray:
    out, _ = _run(inputs, trace=False)
    return out
